# revision 21
# baseline (speedup 1.0000x reference)
"""2-layer GCN (GCNConv+relu x2, linear head) on 8 Trainium2 NeuronCores.

v5 strategy (graph/data parallel per the sharding hint):
  - Nodes sharded across 8 cores by id; edges partitioned by destination.
  - Per core, destinations are packed into two HALVES of B_FIX/2 blocks
    each; the 4 gather streams are (source half, source parity), where a
    node's half is fixed a priori by its position within its core. Slot
    tables are laid out HALF-MAJOR across cores ([c0A..c7A, c0B..c7B]) so
    each stream's gather window is contiguous and fits int16 pair indices.
  - Layer 1 gathers raw dinv-scaled x rows (256B = 128 feats bf16) from a
    replicated half-major pair table in HBM; aggregation commutes with
    W1+W2, which are applied per destination block AFTER the scatter-add.
  - The inter-layer AllGather is SPLIT per half: AG_A is issued mid-layer-1
    (its inputs -- the first half of the staging table -- are already
    complete), AG_B at the end; layer-2 A-stream gathers start immediately
    after layer 1 drains, overlapping AG_B (software-pipelined emission).
  - Scatter-add per destination block via PE matmuls: lhsT = gathered
    messages, rhs = selection matrix S with dinv[dst] folded in, built
    host-side and streamed from HBM. Self-loop terms enter via diag(dinv)
    matmuls. The AllGathered staging tables are pair-HBM Shared.
"""

import numpy as np

import concourse.bass as bass
import concourse.mybir as mybir
import concourse.tile as tile
from concourse import bacc
from concourse import bass_utils

import ml_dtypes

F32 = mybir.dt.float32
BF16 = mybir.dt.bfloat16
I16 = mybir.dt.int16
NP_BF16 = ml_dtypes.bfloat16


class Cfg:
    def __init__(self, n_nodes, in_feat, hidden, n_classes, n_cores,
                 n_c, blk, kcol, b_fix, c_batch):
        self.N = n_nodes
        self.IN_FEAT = in_feat
        self.HIDDEN = hidden
        self.N_CLASSES = n_classes
        self.NC = n_cores
        self.N_C = n_c                    # nodes per core (id // N_C)
        assert n_c * n_cores >= n_nodes
        self.BLK = blk                    # dsts per block
        self.KCOL = kcol                  # 128-columns per (block, stream)
        self.CAP = kcol * 128             # edge slots per (block, stream)
        self.B_FIX = b_fix                # blocks per core
        assert b_fix % 2 == 0
        self.B_HALF = b_fix // 2          # blocks per (core, half)
        self.NQ = 4                       # streams = (source half, parity)
        self.SLOTS_C = b_fix * blk        # table slots per core
        self.SLOTS_H = self.SLOTS_C // 2  # slots per (core, half)
        assert self.SLOTS_C % 128 == 0
        self.NT = self.SLOTS_C // 128     # node tiles per core
        assert self.NT % 4 == 0
        self.TABLE_N = n_cores * self.SLOTS_C
        self.WIN = self.TABLE_N // 2      # slots per half-window
        self.PAIRS_W = self.WIN // 2      # pair rows per window
        assert self.PAIRS_W <= 32767      # int16 pair-index range
        self.COLS_Q = b_fix * kcol        # gather columns per stream
        self.C_BATCH = c_batch            # columns per gather call
        assert c_batch % kcol == 0 and self.COLS_Q % c_batch == 0
        self.N_BATCH = self.COLS_Q // c_batch
        self.BPB = c_batch // kcol        # blocks per batch
        assert self.BPB % 2 == 0
        self.N_CH = n_c // 2              # nodes per (core, half)


CFG_FULL = Cfg(n_nodes=100000, in_feat=128, hidden=64, n_classes=16,
               n_cores=8, n_c=12544, blk=64, kcol=2, b_fix=224, c_batch=28)

AG_A_AFTER_BATCH = 13      # emit AG_A after this layer-1 batch
L2_PROLOGUE = 3            # layer-2 A-stream gather batches before AG_B


# ---------------------------------------------------------------------------
# Host-side preprocessing: graph restructuring only.
# ---------------------------------------------------------------------------

def preprocess(cfg, x, edge_index, W1, b1, W2, b2, Wl, bl):
    N, NC, N_C = cfg.N, cfg.NC, cfg.N_C
    src = np.asarray(edge_index[0]).astype(np.int64)
    dst = np.asarray(edge_index[1]).astype(np.int64)
    x = np.asarray(x, dtype=np.float32)

    deg = np.bincount(dst, minlength=N).astype(np.float32) + 1.0
    dinv = (1.0 / np.sqrt(deg)).astype(np.float32)

    # stream of each edge: (source half, source parity)
    q_of = ((src % N_C) >= cfg.N_CH).astype(np.int64) * 2 + (src & 1)

    # per-(node, q) incoming edge counts
    degq = np.bincount(dst * cfg.NQ + q_of, minlength=N * cfg.NQ)\
             .reshape(N, cfg.NQ)

    # --- per-(core,half) FFD packing of dsts into blocks ---
    half = cfg.BLK // 2
    slot_of = np.full(NC * N_C, -1, dtype=np.int64)
    node_of_slot = np.full(cfg.TABLE_N, -1, dtype=np.int64)
    for c in range(NC):
        for hf in range(2):
            lo = c * N_C + hf * cfg.N_CH
            hi = min(lo + cfg.N_CH, N)
            if hi <= lo:
                continue
            b_base = hf * cfg.B_HALF
            dq = degq[lo:hi]
            par = (np.arange(lo, hi) & 1)
            order = np.argsort(-dq.max(axis=1), kind="stable")
            accs = np.zeros((cfg.B_HALF, cfg.NQ), dtype=np.int64)
            cnts = np.zeros((cfg.B_HALF, 2), dtype=np.int64)
            nopen = 1
            for j in order:
                v = dq[j]
                p = par[j]
                fits = (cnts[:nopen, p] < half) & \
                       np.all(accs[:nopen] + v <= cfg.CAP, axis=1)
                w = np.flatnonzero(fits)
                if w.size == 0:
                    assert nopen < cfg.B_HALF, \
                        f"core {c} half {hf}: packing exceeds " \
                        f"{cfg.B_HALF} blocks"
                    b = nopen
                    nopen += 1
                else:
                    b = int(w[0])
                g = lo + j
                s = (c * cfg.SLOTS_C + (b_base + b) * cfg.BLK +
                     2 * cnts[b, p] + p)
                slot_of[g] = s
                node_of_slot[s] = g
                accs[b] += v
                cnts[b, p] += 1

    slot_of = slot_of[:N]

    # --- per-core edge streams ---
    e_core = dst // N_C
    s_slot = slot_of[src]
    d_slot_l = slot_of[dst] - e_core * cfg.SLOTS_C
    e_b = d_slot_l // cfg.BLK
    e_r = d_slot_l % cfg.BLK

    # half-major position of a global slot
    g_all = np.arange(cfg.TABLE_N)
    g_c = g_all // cfg.SLOTS_C
    g_sl = g_all % cfg.SLOTS_C
    g_hf = (g_sl >= cfg.SLOTS_H).astype(np.int64)
    pos_of_slot = (g_hf * cfg.WIN + g_c * cfg.SLOTS_H +
                   (g_sl - g_hf * cfg.SLOTS_H))

    P_Q = cfg.B_FIX * cfg.CAP            # positions per stream
    idx_all = np.zeros((NC, cfg.NQ, P_Q), dtype=np.int16)
    dl_all = np.full((NC, cfg.NQ, P_Q), 255, dtype=np.int64)

    order2 = np.lexsort((e_b, q_of, e_core))
    es_c, eq_c, eb_c = e_core[order2], q_of[order2], e_b[order2]
    grp = (es_c * cfg.NQ + eq_c) * cfg.B_FIX + eb_c
    _, start_idx, cnt_grp = np.unique(grp, return_index=True,
                                      return_counts=True)
    rank = np.arange(grp.size) - np.repeat(start_idx, cnt_grp)
    assert rank.max(initial=0) < cfg.CAP, "stream capacity overflow"
    pos = eb_c * cfg.CAP + rank
    idx_val = ((pos_of_slot[s_slot[order2]] % cfg.WIN) // 2)\
        .astype(np.int16)
    idx_all[es_c, eq_c, pos] = idx_val
    dl_all[es_c, eq_c, pos] = e_r[order2]

    # wrapped int16 layout: position i -> [i%16, i//16], replicated x8
    idx_w = idx_all.reshape(NC, cfg.NQ, -1, 16).transpose(0, 1, 3, 2)
    idx_dev = np.ascontiguousarray(np.tile(idx_w, (1, 1, 8, 1)))

    # --- per-slot node data: x pre-scaled by dinv, slot-ordered ---
    valid = node_of_slot >= 0
    xe = np.zeros((cfg.TABLE_N, cfg.IN_FEAT), dtype=np.float32)
    xe[valid] = x[node_of_slot[valid]] * dinv[node_of_slot[valid]][:, None]
    dinv_s = np.zeros(cfg.TABLE_N, dtype=np.float32)
    dinv_s[valid] = dinv[node_of_slot[valid]]

    # layer-1 gather table in HALF-MAJOR order, pair rows [TN//2, 2*IF]
    xe_hm = np.zeros_like(xe)
    xe_hm[pos_of_slot] = xe
    xe_pair = np.ascontiguousarray(
        xe_hm.reshape(cfg.TABLE_N // 2, 2 * cfg.IN_FEAT)).astype(NP_BF16)

    # S matrices with dinv[dst] folded in: per (core, stream)
    # dev layout [128, COLS_Q*BLK]: S[p, col*BLK + j] for position col*128+p
    S_dev = []
    for c in range(NC):
        r = dl_all[c]                                    # [NQ, P_Q]
        bidx = np.arange(P_Q) // cfg.CAP                 # block of position
        v = r < 255
        dloc = np.where(v, bidx[None, :] * cfg.BLK + r, 0)
        val = dinv_s[c * cfg.SLOTS_C + dloc] * v         # [NQ, P_Q]
        S = np.zeros((cfg.NQ, P_Q, cfg.BLK), dtype=np.float32)
        qq, pp = np.nonzero(v)
        S[qq, pp, r[qq, pp]] = val[qq, pp]
        S = S.reshape(cfg.NQ, cfg.COLS_Q, 128, cfg.BLK)\
             .transpose(0, 2, 1, 3)\
             .reshape(cfg.NQ, 128, cfg.COLS_Q * cfg.BLK)
        S_dev.append(np.ascontiguousarray(S.astype(NP_BF16)))

    # diag(dinv) tiles for self-loop terms: [128, NT*BLK]
    diagd_all = []
    for c in range(NC):
        dv = dinv_s[c * cfg.SLOTS_C:(c + 1) * cfg.SLOTS_C]\
            .reshape(cfg.NT, 128)
        dgd = np.zeros((128, cfg.NT * cfg.BLK), dtype=np.float32)
        t_i = np.arange(cfg.NT)[:, None]
        j_i = np.arange(cfg.BLK)[None, :]
        dgd[j_i, t_i * cfg.BLK + j_i] = dv[:, :64]
        dgd[64 + j_i, t_i * cfg.BLK + j_i] = dv[:, 64:]
        diagd_all.append(dgd.astype(NP_BF16))

    # own xe rows, slot-tile-major for L1 self lhsT: [128, NT*IN_FEAT]
    xe_own_all = []
    for c in range(NC):
        xo = xe[c * cfg.SLOTS_C:(c + 1) * cfg.SLOTS_C]\
            .reshape(cfg.NT, 128, cfg.IN_FEAT).transpose(1, 0, 2)\
            .reshape(128, cfg.NT * cfg.IN_FEAT)
        xe_own_all.append(np.ascontiguousarray(xo).astype(NP_BF16))

    W1 = np.asarray(W1, np.float32).astype(NP_BF16)
    W2 = np.asarray(W2, np.float32).astype(NP_BF16)
    Wl = np.asarray(Wl, np.float32).astype(NP_BF16)
    b1 = np.asarray(b1, np.float32)
    b2 = np.asarray(b2, np.float32)
    bl = np.asarray(bl, np.float32)

    in_maps = []
    for c in range(NC):
        dv = dinv_s[c * cfg.SLOTS_C:(c + 1) * cfg.SLOTS_C]
        m = {
            "xep": xe_pair,
            "xeown": xe_own_all[c],
            "diagd": diagd_all[c],
            "w1": W1, "w2": W2, "wl": Wl,
            "b1c": b1.reshape(-1, 1),
            "b2c": b2.reshape(-1, 1),
            "blrep": np.tile(bl[None, :], (128, 1)),
            "dinvn": np.ascontiguousarray(dv.reshape(cfg.NT, 128).T),
        }
        for q in range(cfg.NQ):
            m[f"idx{q}"] = idx_dev[c, q]
            m[f"sm{q}"] = S_dev[c][q]
        in_maps.append(m)

    return in_maps, node_of_slot


def assemble_output(cfg, results, node_of_slot):
    out = np.zeros((cfg.N, cfg.N_CLASSES), dtype=np.float32)
    for c, r in enumerate(results):
        lg = r["logits"].reshape(128, cfg.NT, cfg.N_CLASSES)
        sl = node_of_slot[c * cfg.SLOTS_C:(c + 1) * cfg.SLOTS_C]\
            .reshape(cfg.NT, 128)
        for t in range(cfg.NT):
            v = sl[t] >= 0
            out[sl[t][v]] = lg[v, t, :]
    return out


# ---------------------------------------------------------------------------
# Device program
# ---------------------------------------------------------------------------

def build_program(cfg):
    nc = bacc.Bacc("TRN2", target_bir_lowering=False, debug=False,
                   num_devices=cfg.NC, num_swdge_queues=4)
    H, NT, BLK, IF = cfg.HIDDEN, cfg.NT, cfg.BLK, cfg.IN_FEAT
    PAIRS_W = cfg.PAIRS_W

    xep_d = nc.dram_tensor("xep", [cfg.TABLE_N // 2, 2 * IF], BF16,
                           kind="ExternalInput")
    xeown_d = nc.dram_tensor("xeown", [128, NT * IF], BF16,
                             kind="ExternalInput")
    diagd_d = nc.dram_tensor("diagd", [128, NT * BLK], BF16,
                             kind="ExternalInput")
    w1_d = nc.dram_tensor("w1", [IF, H], BF16, kind="ExternalInput")
    w2_d = nc.dram_tensor("w2", [H, H], BF16, kind="ExternalInput")
    wl_d = nc.dram_tensor("wl", [H, cfg.N_CLASSES], BF16,
                          kind="ExternalInput")
    b1c_d = nc.dram_tensor("b1c", [H, 1], F32, kind="ExternalInput")
    b2c_d = nc.dram_tensor("b2c", [H, 1], F32, kind="ExternalInput")
    blrep_d = nc.dram_tensor("blrep", [128, cfg.N_CLASSES], F32,
                             kind="ExternalInput")
    dinvn_d = nc.dram_tensor("dinvn", [128, NT], F32, kind="ExternalInput")
    idx_d = [nc.dram_tensor(f"idx{q}", [128, cfg.COLS_Q * 8], I16,
                            kind="ExternalInput") for q in range(cfg.NQ)]
    sm_d = [nc.dram_tensor(f"sm{q}", [128, cfg.COLS_Q * BLK], BF16,
                           kind="ExternalInput") for q in range(cfg.NQ)]
    logits_d = nc.dram_tensor("logits", [128, NT * cfg.N_CLASSES], F32,
                              kind="ExternalOutput")

    rg = [list(range(cfg.NC))]

    with tile.TileContext(nc) as tc:
        with tc.tile_pool(name="const", bufs=1) as cpool, \
             tc.tile_pool(name="dram", bufs=1, space="DRAM") as dpool, \
             tc.tile_pool(name="hp", bufs=3) as hpool:

            hs2_t = dpool.tile([cfg.SLOTS_C, H], BF16, tag="hs2")
            tabA_t = dpool.tile([PAIRS_W, 2 * H], BF16, tag="tabA",
                                addr_space="Shared")
            tabB_t = dpool.tile([PAIRS_W, 2 * H], BF16, tag="tabB",
                                addr_space="Shared")

            def cload(dram, shape, dt, tag):
                t = cpool.tile(shape, dt, name=f"c_{tag}", tag=tag)
                nc.sync.dma_start(out=t[:], in_=dram[:, :])
                return t

            w1_s = cload(w1_d, [IF, H], BF16, "w1")
            w2_s = cload(w2_d, [H, H], BF16, "w2")
            wl_s = cload(wl_d, [H, cfg.N_CLASSES], BF16, "wl")
            b1c_s = cload(b1c_d, [H, 1], F32, "b1c")
            b2c_s = cload(b2c_d, [H, 1], F32, "b2c")
            blrep_s = cload(blrep_d, [128, cfg.N_CLASSES], F32, "blrep")
            dinvn_s = cload(dinvn_d, [128, NT], F32, "dinvn")
            diagd_s = cload(diagd_d, [128, NT * BLK], BF16, "diagd")

            hs2own_s = cpool.tile([128, NT * H], BF16, tag="hs2own")
            stageL_s = cpool.tile([128, NT * cfg.N_CLASSES], F32, tag="stgL")

            with tc.tile_pool(name="sp", bufs=2) as spool, \
                 tc.tile_pool(name="ip", bufs=4) as ipool, \
                 tc.tile_pool(name="pp", bufs=2, space="PSUM") as pp, \
                 tc.tile_pool(name="pc", bufs=2, space="PSUM") as pc:

                gt = {}    # (layer, i, q) -> msg tile
                st = {}    # (i, q) -> S tile

                it = {}    # (layer, i, q) -> idx tile

                def emit_idx(layer, i, q):
                    idx_t = ipool.tile([128, cfg.C_BATCH * 8], I16,
                                       name=f"ix{layer}_{i}_{q}",
                                       tag=f"ix{q}")
                    nc.scalar.dma_start(
                        out=idx_t[:],
                        in_=idx_d[q][:, i * cfg.C_BATCH * 8:
                                     (i + 1) * cfg.C_BATCH * 8])
                    it[(layer, i, q)] = idx_t

                def emit_gather(mpool, layer, i, q, chunks=1):
                    idx_t = it.pop((layer, i, q))
                    par = q % 2
                    cb = cfg.C_BATCH // chunks
                    if layer == 1:
                        wlo = (q // 2) * PAIRS_W
                        msg_t = mpool.tile([128, cfg.C_BATCH, IF], BF16,
                                           name=f"m1_{i}_{q}",
                                           tag=f"msg{q}")
                        for ch in range(chunks):
                            nc.gpsimd.dma_gather(
                                out_ap=msg_t[:, ch * cb:(ch + 1) * cb, :],
                                in_ap=xep_d[wlo:wlo + PAIRS_W,
                                            par * IF:(par + 1) * IF],
                                idxs_ap=idx_t[:, ch * cb * 8:
                                              (ch + 1) * cb * 8],
                                num_idxs=cb * 128,
                                num_idxs_reg=cb * 128,
                                elem_size=IF, elem_step=2 * IF,
                                queue_num=q, single_packet=False)
                    else:
                        tab = tabA_t if q < 2 else tabB_t
                        msg_t = mpool.tile([128, cfg.C_BATCH, 2 * H], BF16,
                                           name=f"m2_{i}_{q}",
                                           tag=f"msg{q}")
                        for ch in range(chunks):
                            nc.gpsimd.dma_gather(
                                out_ap=msg_t[:, ch * cb:(ch + 1) * cb, :],
                                in_ap=tab[:, :],
                                idxs_ap=idx_t[:, ch * cb * 8:
                                              (ch + 1) * cb * 8],
                                num_idxs=cb * 128,
                                num_idxs_reg=cb * 128,
                                elem_size=2 * H, queue_num=q,
                                single_packet=False)
                    gt[(layer, i, q)] = msg_t

                def emit_S(i, q):
                    S_t = spool.tile([128, cfg.C_BATCH, BLK], BF16,
                                     name=f"S_{i}_{q}", tag=f"S{q}")
                    nc.sync.dma_start(
                        out=S_t[:],
                        in_=sm_d[q][:, i * cfg.C_BATCH * BLK:
                                    (i + 1) * cfg.C_BATCH * BLK]
                        .rearrange("p (c f) -> p c f", f=BLK))
                    st[(i, q)] = S_t

                pair = {}

                def emit_consumers(layer, i, xeown_s):
                    msgs = [gt.pop((layer, i, q)) for q in range(cfg.NQ)]
                    Ss = [st.pop((i, q)) for q in range(cfg.NQ)]
                    for bb in range(cfg.BPB):
                        b = i * cfg.BPB + bb
                        t = b // 2
                        h = b % 2
                        ho = h * 64
                        pfm_full = pp.tile([128, BLK], F32,
                                           name=f"pfm{layer}_{b}",
                                           tag="fm")
                        if layer == 1:
                            pfm = pfm_full
                            nc.tensor.matmul(
                                out=pfm[:],
                                lhsT=xeown_s[ho:ho + 64,
                                             t * IF:(t + 1) * IF],
                                rhs=diagd_s[ho:ho + 64,
                                            t * BLK:(t + 1) * BLK],
                                start=True, stop=False)
                        else:
                            pfm = pfm_full[:H, :]
                            nc.tensor.matmul(
                                out=pfm[:],
                                lhsT=hs2own_s[ho:ho + 64,
                                              t * H:(t + 1) * H],
                                rhs=diagd_s[ho:ho + 64,
                                            t * BLK:(t + 1) * BLK],
                                start=True, stop=False)
                        for q in range(cfg.NQ):
                            par = q % 2
                            for k in range(cfg.KCOL):
                                lc = bb * cfg.KCOL + k
                                last = (q == cfg.NQ - 1 and
                                        k == cfg.KCOL - 1)
                                if layer == 1:
                                    lhsT_m = msgs[q][:, lc:lc + 1, :]\
                                        .rearrange("p c f -> p (c f)")
                                else:
                                    lhsT_m = msgs[q][:, lc:lc + 1,
                                                     par * H:(par + 1) * H]\
                                        .rearrange("p c f -> p (c f)")
                                rhs_S = Ss[q][:, lc:lc + 1, :]\
                                    .rearrange("p c f -> p (c f)")
                                nc.tensor.matmul(
                                    out=pfm[:], lhsT=lhsT_m, rhs=rhs_S,
                                    start=False, stop=last)
                        if layer == 1:
                            pf_s = hpool.tile([128, BLK], BF16,
                                              name=f"pf1s_{b}", tag="pf1s")
                            nc.vector.tensor_copy(out=pf_s[:], in_=pfm[:])
                            pW = pp.tile([H, BLK], F32, name=f"pW_{b}",
                                         tag="pW")
                            nc.tensor.matmul(
                                out=pW[:], lhsT=w1_s[:], rhs=pf_s[:],
                                start=True, stop=True)
                            hr_t = hpool.tile([H, BLK], BF16,
                                              name=f"hr1_{b}", tag="hr1")
                            nc.scalar.activation(
                                out=hr_t[:], in_=pW[:],
                                func=mybir.ActivationFunctionType.Relu,
                                bias=b1c_s[:])
                            if h == 0:
                                pair["p2"] = pc.tile([128, H], F32,
                                                     name=f"p2_{b}",
                                                     tag="pair")
                            p2 = pair["p2"]
                            nc.tensor.matmul(
                                out=p2[ho:ho + 64, :], lhsT=hr_t[:],
                                rhs=w2_s[:], start=True, stop=True,
                                tile_position=(0, ho))
                            if h == 1:
                                nc.vector.tensor_scalar_mul(
                                    out=hs2own_s[:, t * H:(t + 1) * H],
                                    in0=p2[:],
                                    scalar1=dinvn_s[:, t:t + 1])
                                nc.sync.dma_start(
                                    out=hs2_t[t * 128:(t + 1) * 128, :],
                                    in_=hs2own_s[:, t * H:(t + 1) * H])
                        else:
                            hr_t = hpool.tile([H, BLK], BF16,
                                              name=f"hr2_{b}", tag="hr2")
                            nc.scalar.activation(
                                out=hr_t[:], in_=pfm[:],
                                func=mybir.ActivationFunctionType.Relu,
                                bias=b2c_s[:])
                            if h == 0:
                                pair["pl"] = pc.tile(
                                    [128, cfg.N_CLASSES], F32,
                                    name=f"pl_{b}", tag="pl")
                            pl = pair["pl"]
                            nc.tensor.matmul(
                                out=pl[ho:ho + 64, :], lhsT=hr_t[:],
                                rhs=wl_s[:], start=True, stop=True,
                                tile_position=(0, ho))
                            if h == 1:
                                nCL = cfg.N_CLASSES
                                nc.vector.tensor_tensor(
                                    out=stageL_s[:, t * nCL:(t + 1) * nCL],
                                    in0=pl[:], in1=blrep_s[:],
                                    op=mybir.AluOpType.add)

                # ---- both layers, single AllGather between ----
                with tc.tile_pool(name="xo", bufs=1) as xopool, \
                     tc.tile_pool(name="mp", bufs=3) as mp:
                    xeown_s = xopool.tile([128, NT * IF], BF16,
                                          name="xeown_s", tag="xeown")
                    nc.sync.dma_start(out=xeown_s[:], in_=xeown_d[:, :])
                    for q in range(cfg.NQ):
                        emit_idx(1, 0, q)
                    for i in range(cfg.N_BATCH):
                        for q in range(cfg.NQ):
                            emit_gather(mp, 1, i, q)
                            if i + 1 < cfg.N_BATCH:
                                emit_idx(1, i + 1, q)
                            emit_S(i, q)
                        emit_consumers(1, i, xeown_s)
                    for q in range(cfg.NQ):
                        emit_idx(2, 0, q)
                    nc.gpsimd.collective_compute(
                        "AllGather", mybir.AluOpType.bypass,
                        replica_groups=rg,
                        ins=[hs2_t[0:cfg.SLOTS_H, :].opt()],
                        outs=[tabA_t.opt()])
                    nc.gpsimd.collective_compute(
                        "AllGather", mybir.AluOpType.bypass,
                        replica_groups=rg,
                        ins=[hs2_t[cfg.SLOTS_H:cfg.SLOTS_C, :].opt()],
                        outs=[tabB_t.opt()])
                    for i in range(cfg.N_BATCH):
                        for q in range(cfg.NQ):
                            emit_gather(mp, 2, i, q)
                            if i + 1 < cfg.N_BATCH:
                                emit_idx(2, i + 1, q)
                            emit_S(i, q)
                        emit_consumers(2, i, None)

            nc.sync.dma_start(out=logits_d[:, :], in_=stageL_s[:])

    nc.compile()
    return nc


_PROGRAM_CACHE = {}


def get_program(cfg):
    key = id(cfg)
    if key not in _PROGRAM_CACHE:
        _PROGRAM_CACHE[key] = build_program(cfg)
    return _PROGRAM_CACHE[key]


def run(cfg, inputs, trace=False):
    in_maps, node_of_slot = preprocess(cfg, **inputs)
    nc = get_program(cfg)
    res = bass_utils.run_bass_kernel_spmd(
        nc, in_maps, core_ids=list(range(cfg.NC)), trace=trace)
    out = assemble_output(cfg, res.results, node_of_slot)
    return out, res


def kernel(**inputs) -> np.ndarray:
    out, _ = run(CFG_FULL, inputs)
    return out


# revision 23
# speedup vs baseline: 1.0067x; 1.0067x over previous
"""2-layer GCN (GCNConv+relu x2, linear head) on 8 Trainium2 NeuronCores.

v5 strategy (graph/data parallel per the sharding hint):
  - Nodes sharded across 8 cores by id; edges partitioned by destination.
  - Per core, destinations are packed into two HALVES of B_FIX/2 blocks
    each; the 4 gather streams are (source half, source parity), where a
    node's half is fixed a priori by its position within its core. Slot
    tables are laid out HALF-MAJOR across cores ([c0A..c7A, c0B..c7B]) so
    each stream's gather window is contiguous and fits int16 pair indices.
  - Layer 1 gathers raw dinv-scaled x rows (256B = 128 feats bf16) from a
    replicated half-major pair table in HBM; aggregation commutes with
    W1+W2, which are applied per destination block AFTER the scatter-add.
  - The inter-layer AllGather is SPLIT per half: AG_A is issued mid-layer-1
    (its inputs -- the first half of the staging table -- are already
    complete), AG_B at the end; layer-2 A-stream gathers start immediately
    after layer 1 drains, overlapping AG_B (software-pipelined emission).
  - Scatter-add per destination block via PE matmuls: lhsT = gathered
    messages, rhs = selection matrix S with dinv[dst] folded in, built
    host-side and streamed from HBM. Self-loop terms enter via diag(dinv)
    matmuls. The AllGathered staging tables are pair-HBM Shared.
"""

import numpy as np

import concourse.bass as bass
import concourse.mybir as mybir
import concourse.tile as tile
from concourse import bacc
from concourse import bass_utils

import ml_dtypes

F32 = mybir.dt.float32
BF16 = mybir.dt.bfloat16
I16 = mybir.dt.int16
NP_BF16 = ml_dtypes.bfloat16


class Cfg:
    def __init__(self, n_nodes, in_feat, hidden, n_classes, n_cores,
                 n_c, blk, kcol, b_fix, c_batch):
        self.N = n_nodes
        self.IN_FEAT = in_feat
        self.HIDDEN = hidden
        self.N_CLASSES = n_classes
        self.NC = n_cores
        self.N_C = n_c                    # nodes per core (id // N_C)
        assert n_c * n_cores >= n_nodes
        self.BLK = blk                    # dsts per block
        self.KCOL = kcol                  # 128-columns per (block, stream)
        self.CAP = kcol * 128             # edge slots per (block, stream)
        self.B_FIX = b_fix                # blocks per core
        assert b_fix % 2 == 0
        self.B_HALF = b_fix // 2          # blocks per (core, half)
        self.NQ = 4                       # streams = (source half, parity)
        self.SLOTS_C = b_fix * blk        # table slots per core
        self.SLOTS_H = self.SLOTS_C // 2  # slots per (core, half)
        assert self.SLOTS_C % 128 == 0
        self.NT = self.SLOTS_C // 128     # node tiles per core
        assert self.NT % 4 == 0
        self.TABLE_N = n_cores * self.SLOTS_C
        self.WIN = self.TABLE_N // 2      # slots per half-window
        self.PAIRS_W = self.WIN // 2      # pair rows per window
        assert self.PAIRS_W <= 32767      # int16 pair-index range
        self.COLS_Q = b_fix * kcol        # gather columns per stream
        self.C_BATCH = c_batch            # columns per gather call
        assert c_batch % kcol == 0 and self.COLS_Q % c_batch == 0
        self.N_BATCH = self.COLS_Q // c_batch
        self.BPB = c_batch // kcol        # blocks per batch
        assert self.BPB % 2 == 0
        self.N_CH = n_c // 2              # nodes per (core, half)


CFG_FULL = Cfg(n_nodes=100000, in_feat=128, hidden=64, n_classes=16,
               n_cores=8, n_c=12544, blk=64, kcol=2, b_fix=224, c_batch=28)

AG_A_AFTER_BATCH = 13      # emit AG_A after this layer-1 batch
L2_PROLOGUE = 3            # layer-2 A-stream gather batches before AG_B


# ---------------------------------------------------------------------------
# Host-side preprocessing: graph restructuring only.
# ---------------------------------------------------------------------------

def preprocess(cfg, x, edge_index, W1, b1, W2, b2, Wl, bl):
    N, NC, N_C = cfg.N, cfg.NC, cfg.N_C
    src = np.asarray(edge_index[0]).astype(np.int64)
    dst = np.asarray(edge_index[1]).astype(np.int64)
    x = np.asarray(x, dtype=np.float32)

    deg = np.bincount(dst, minlength=N).astype(np.float32) + 1.0
    dinv = (1.0 / np.sqrt(deg)).astype(np.float32)

    # stream of each edge: (source half, source parity)
    q_of = ((src % N_C) >= cfg.N_CH).astype(np.int64) * 2 + (src & 1)

    # per-(node, q) incoming edge counts
    degq = np.bincount(dst * cfg.NQ + q_of, minlength=N * cfg.NQ)\
             .reshape(N, cfg.NQ)

    # --- per-(core,half) FFD packing of dsts into blocks ---
    half = cfg.BLK // 2
    slot_of = np.full(NC * N_C, -1, dtype=np.int64)
    node_of_slot = np.full(cfg.TABLE_N, -1, dtype=np.int64)
    for c in range(NC):
        for hf in range(2):
            lo = c * N_C + hf * cfg.N_CH
            hi = min(lo + cfg.N_CH, N)
            if hi <= lo:
                continue
            b_base = hf * cfg.B_HALF
            dq = degq[lo:hi]
            par = (np.arange(lo, hi) & 1)
            order = np.argsort(-dq.max(axis=1), kind="stable")
            accs = np.zeros((cfg.B_HALF, cfg.NQ), dtype=np.int64)
            cnts = np.zeros((cfg.B_HALF, 2), dtype=np.int64)
            nopen = 1
            for j in order:
                v = dq[j]
                p = par[j]
                fits = (cnts[:nopen, p] < half) & \
                       np.all(accs[:nopen] + v <= cfg.CAP, axis=1)
                w = np.flatnonzero(fits)
                if w.size == 0:
                    assert nopen < cfg.B_HALF, \
                        f"core {c} half {hf}: packing exceeds " \
                        f"{cfg.B_HALF} blocks"
                    b = nopen
                    nopen += 1
                else:
                    b = int(w[0])
                g = lo + j
                s = (c * cfg.SLOTS_C + (b_base + b) * cfg.BLK +
                     2 * cnts[b, p] + p)
                slot_of[g] = s
                node_of_slot[s] = g
                accs[b] += v
                cnts[b, p] += 1

    slot_of = slot_of[:N]

    # --- per-core edge streams ---
    e_core = dst // N_C
    s_slot = slot_of[src]
    d_slot_l = slot_of[dst] - e_core * cfg.SLOTS_C
    e_b = d_slot_l // cfg.BLK
    e_r = d_slot_l % cfg.BLK

    # half-major position of a global slot
    g_all = np.arange(cfg.TABLE_N)
    g_c = g_all // cfg.SLOTS_C
    g_sl = g_all % cfg.SLOTS_C
    g_hf = (g_sl >= cfg.SLOTS_H).astype(np.int64)
    pos_of_slot = (g_hf * cfg.WIN + g_c * cfg.SLOTS_H +
                   (g_sl - g_hf * cfg.SLOTS_H))

    P_Q = cfg.B_FIX * cfg.CAP            # positions per stream
    idx_all = np.zeros((NC, cfg.NQ, P_Q), dtype=np.int16)
    dl_all = np.full((NC, cfg.NQ, P_Q), 255, dtype=np.int64)

    order2 = np.lexsort((e_b, q_of, e_core))
    es_c, eq_c, eb_c = e_core[order2], q_of[order2], e_b[order2]
    grp = (es_c * cfg.NQ + eq_c) * cfg.B_FIX + eb_c
    _, start_idx, cnt_grp = np.unique(grp, return_index=True,
                                      return_counts=True)
    rank = np.arange(grp.size) - np.repeat(start_idx, cnt_grp)
    assert rank.max(initial=0) < cfg.CAP, "stream capacity overflow"
    pos = eb_c * cfg.CAP + rank
    idx_val = ((pos_of_slot[s_slot[order2]] % cfg.WIN) // 2)\
        .astype(np.int16)
    idx_all[es_c, eq_c, pos] = idx_val
    dl_all[es_c, eq_c, pos] = e_r[order2]

    # wrapped int16 layout: position i -> [i%16, i//16], replicated x8
    idx_w = idx_all.reshape(NC, cfg.NQ, -1, 16).transpose(0, 1, 3, 2)
    idx_dev = np.ascontiguousarray(np.tile(idx_w, (1, 1, 8, 1)))

    # --- per-slot node data: x pre-scaled by dinv, slot-ordered ---
    valid = node_of_slot >= 0
    xe = np.zeros((cfg.TABLE_N, cfg.IN_FEAT), dtype=np.float32)
    xe[valid] = x[node_of_slot[valid]] * dinv[node_of_slot[valid]][:, None]
    dinv_s = np.zeros(cfg.TABLE_N, dtype=np.float32)
    dinv_s[valid] = dinv[node_of_slot[valid]]

    # layer-1 gather table in HALF-MAJOR order, pair rows [TN//2, 2*IF]
    xe_hm = np.zeros_like(xe)
    xe_hm[pos_of_slot] = xe
    xe_pair = np.ascontiguousarray(
        xe_hm.reshape(cfg.TABLE_N // 2, 2 * cfg.IN_FEAT)).astype(NP_BF16)

    # S matrices with dinv[dst] folded in: per (core, stream)
    # dev layout [128, COLS_Q*BLK]: S[p, col*BLK + j] for position col*128+p
    S_dev = []
    for c in range(NC):
        r = dl_all[c]                                    # [NQ, P_Q]
        bidx = np.arange(P_Q) // cfg.CAP                 # block of position
        v = r < 255
        dloc = np.where(v, bidx[None, :] * cfg.BLK + r, 0)
        val = dinv_s[c * cfg.SLOTS_C + dloc] * v         # [NQ, P_Q]
        S = np.zeros((cfg.NQ, P_Q, cfg.BLK), dtype=np.float32)
        qq, pp = np.nonzero(v)
        S[qq, pp, r[qq, pp]] = val[qq, pp]
        S = S.reshape(cfg.NQ, cfg.COLS_Q, 128, cfg.BLK)\
             .transpose(0, 2, 1, 3)\
             .reshape(cfg.NQ, 128, cfg.COLS_Q * cfg.BLK)
        S_dev.append(np.ascontiguousarray(S.astype(NP_BF16)))

    # diag(dinv) tiles for self-loop terms: [128, NT*BLK]
    diagd_all = []
    for c in range(NC):
        dv = dinv_s[c * cfg.SLOTS_C:(c + 1) * cfg.SLOTS_C]\
            .reshape(cfg.NT, 128)
        dgd = np.zeros((128, cfg.NT * cfg.BLK), dtype=np.float32)
        t_i = np.arange(cfg.NT)[:, None]
        j_i = np.arange(cfg.BLK)[None, :]
        dgd[j_i, t_i * cfg.BLK + j_i] = dv[:, :64]
        dgd[64 + j_i, t_i * cfg.BLK + j_i] = dv[:, 64:]
        diagd_all.append(dgd.astype(NP_BF16))

    # own xe rows, slot-tile-major for L1 self lhsT: [128, NT*IN_FEAT]
    xe_own_all = []
    for c in range(NC):
        xo = xe[c * cfg.SLOTS_C:(c + 1) * cfg.SLOTS_C]\
            .reshape(cfg.NT, 128, cfg.IN_FEAT).transpose(1, 0, 2)\
            .reshape(128, cfg.NT * cfg.IN_FEAT)
        xe_own_all.append(np.ascontiguousarray(xo).astype(NP_BF16))

    W1 = np.asarray(W1, np.float32).astype(NP_BF16)
    W2 = np.asarray(W2, np.float32).astype(NP_BF16)
    Wl = np.asarray(Wl, np.float32).astype(NP_BF16)
    b1 = np.asarray(b1, np.float32)
    b2 = np.asarray(b2, np.float32)
    bl = np.asarray(bl, np.float32)

    in_maps = []
    for c in range(NC):
        dv = dinv_s[c * cfg.SLOTS_C:(c + 1) * cfg.SLOTS_C]
        m = {
            "xep": xe_pair,
            "xeown": xe_own_all[c],
            "diagd": diagd_all[c],
            "w1": W1, "w2": W2, "wl": Wl,
            "b1c": b1.reshape(-1, 1),
            "b2c": b2.reshape(-1, 1),
            "blrep": np.tile(bl[None, :], (128, 1)),
            "dinvn": np.ascontiguousarray(dv.reshape(cfg.NT, 128).T),
        }
        for q in range(cfg.NQ):
            m[f"idx{q}"] = idx_dev[c, q]
            m[f"sm{q}"] = S_dev[c][q]
        in_maps.append(m)

    return in_maps, node_of_slot


def assemble_output(cfg, results, node_of_slot):
    out = np.zeros((cfg.N, cfg.N_CLASSES), dtype=np.float32)
    for c, r in enumerate(results):
        lg = r["logits"].reshape(128, cfg.NT, cfg.N_CLASSES)
        sl = node_of_slot[c * cfg.SLOTS_C:(c + 1) * cfg.SLOTS_C]\
            .reshape(cfg.NT, 128)
        for t in range(cfg.NT):
            v = sl[t] >= 0
            out[sl[t][v]] = lg[v, t, :]
    return out


# ---------------------------------------------------------------------------
# Device program
# ---------------------------------------------------------------------------

def build_program(cfg):
    nc = bacc.Bacc("TRN2", target_bir_lowering=False, debug=False,
                   num_devices=cfg.NC, num_swdge_queues=4)
    H, NT, BLK, IF = cfg.HIDDEN, cfg.NT, cfg.BLK, cfg.IN_FEAT
    PAIRS_W = cfg.PAIRS_W

    xep_d = nc.dram_tensor("xep", [cfg.TABLE_N // 2, 2 * IF], BF16,
                           kind="ExternalInput")
    xeown_d = nc.dram_tensor("xeown", [128, NT * IF], BF16,
                             kind="ExternalInput")
    diagd_d = nc.dram_tensor("diagd", [128, NT * BLK], BF16,
                             kind="ExternalInput")
    w1_d = nc.dram_tensor("w1", [IF, H], BF16, kind="ExternalInput")
    w2_d = nc.dram_tensor("w2", [H, H], BF16, kind="ExternalInput")
    wl_d = nc.dram_tensor("wl", [H, cfg.N_CLASSES], BF16,
                          kind="ExternalInput")
    b1c_d = nc.dram_tensor("b1c", [H, 1], F32, kind="ExternalInput")
    b2c_d = nc.dram_tensor("b2c", [H, 1], F32, kind="ExternalInput")
    blrep_d = nc.dram_tensor("blrep", [128, cfg.N_CLASSES], F32,
                             kind="ExternalInput")
    dinvn_d = nc.dram_tensor("dinvn", [128, NT], F32, kind="ExternalInput")
    idx_d = [nc.dram_tensor(f"idx{q}", [128, cfg.COLS_Q * 8], I16,
                            kind="ExternalInput") for q in range(cfg.NQ)]
    sm_d = [nc.dram_tensor(f"sm{q}", [128, cfg.COLS_Q * BLK], BF16,
                           kind="ExternalInput") for q in range(cfg.NQ)]
    logits_d = nc.dram_tensor("logits", [128, NT * cfg.N_CLASSES], F32,
                              kind="ExternalOutput")

    rg = [list(range(cfg.NC))]

    with tile.TileContext(nc) as tc:
        with tc.tile_pool(name="const", bufs=1) as cpool, \
             tc.tile_pool(name="dram", bufs=1, space="DRAM") as dpool, \
             tc.tile_pool(name="hp", bufs=3) as hpool:

            hs2_t = dpool.tile([cfg.SLOTS_C, H], BF16, tag="hs2")
            tabA_t = dpool.tile([PAIRS_W, 2 * H], BF16, tag="tabA",
                                addr_space="Shared")
            tabB_t = dpool.tile([PAIRS_W, 2 * H], BF16, tag="tabB",
                                addr_space="Shared")

            def cload(dram, shape, dt, tag):
                t = cpool.tile(shape, dt, name=f"c_{tag}", tag=tag)
                nc.sync.dma_start(out=t[:], in_=dram[:, :])
                return t

            w1_s = cload(w1_d, [IF, H], BF16, "w1")
            w2_s = cload(w2_d, [H, H], BF16, "w2")
            wl_s = cload(wl_d, [H, cfg.N_CLASSES], BF16, "wl")
            b1c_s = cload(b1c_d, [H, 1], F32, "b1c")
            b2c_s = cload(b2c_d, [H, 1], F32, "b2c")
            blrep_s = cload(blrep_d, [128, cfg.N_CLASSES], F32, "blrep")
            dinvn_s = cload(dinvn_d, [128, NT], F32, "dinvn")
            diagd_s = cload(diagd_d, [128, NT * BLK], BF16, "diagd")

            hs2own_s = cpool.tile([128, NT * H], BF16, tag="hs2own")
            stageL_s = cpool.tile([128, NT * cfg.N_CLASSES], F32, tag="stgL")

            with tc.tile_pool(name="sp", bufs=3) as spool, \
                 tc.tile_pool(name="ip", bufs=4) as ipool, \
                 tc.tile_pool(name="pp", bufs=2, space="PSUM") as pp, \
                 tc.tile_pool(name="pc", bufs=2, space="PSUM") as pc:

                gt = {}    # (layer, i, q) -> msg tile
                st = {}    # (i, q) -> S tile

                def emit_gather(mpool, layer, i, q, chunks=1):
                    idx_t = ipool.tile([128, cfg.C_BATCH * 8], I16,
                                       name=f"ix{layer}_{i}_{q}",
                                       tag=f"ix{q}")
                    nc.scalar.dma_start(
                        out=idx_t[:],
                        in_=idx_d[q][:, i * cfg.C_BATCH * 8:
                                     (i + 1) * cfg.C_BATCH * 8])
                    par = q % 2
                    cb = cfg.C_BATCH // chunks
                    if layer == 1:
                        wlo = (q // 2) * PAIRS_W
                        msg_t = mpool.tile([128, cfg.C_BATCH, IF], BF16,
                                           name=f"m1_{i}_{q}",
                                           tag=f"msg{q}")
                        for ch in range(chunks):
                            nc.gpsimd.dma_gather(
                                out_ap=msg_t[:, ch * cb:(ch + 1) * cb, :],
                                in_ap=xep_d[wlo:wlo + PAIRS_W,
                                            par * IF:(par + 1) * IF],
                                idxs_ap=idx_t[:, ch * cb * 8:
                                              (ch + 1) * cb * 8],
                                num_idxs=cb * 128,
                                num_idxs_reg=cb * 128,
                                elem_size=IF, elem_step=2 * IF,
                                queue_num=q, single_packet=False)
                    else:
                        tab = tabA_t if q < 2 else tabB_t
                        msg_t = mpool.tile([128, cfg.C_BATCH, 2 * H], BF16,
                                           name=f"m2_{i}_{q}",
                                           tag=f"msg{q}")
                        for ch in range(chunks):
                            nc.gpsimd.dma_gather(
                                out_ap=msg_t[:, ch * cb:(ch + 1) * cb, :],
                                in_ap=tab[:, :],
                                idxs_ap=idx_t[:, ch * cb * 8:
                                              (ch + 1) * cb * 8],
                                num_idxs=cb * 128,
                                num_idxs_reg=cb * 128,
                                elem_size=2 * H, queue_num=q,
                                single_packet=False)
                    gt[(layer, i, q)] = msg_t

                def emit_S(i, q):
                    S_t = spool.tile([128, cfg.C_BATCH, BLK], BF16,
                                     name=f"S_{i}_{q}", tag=f"S{q}")
                    nc.sync.dma_start(
                        out=S_t[:],
                        in_=sm_d[q][:, i * cfg.C_BATCH * BLK:
                                    (i + 1) * cfg.C_BATCH * BLK]
                        .rearrange("p (c f) -> p c f", f=BLK))
                    st[(i, q)] = S_t

                pair = {}

                def emit_consumers(layer, i, xeown_s):
                    msgs = [gt.pop((layer, i, q)) for q in range(cfg.NQ)]
                    Ss = [st.pop((i, q)) for q in range(cfg.NQ)]
                    for bb in range(cfg.BPB):
                        b = i * cfg.BPB + bb
                        t = b // 2
                        h = b % 2
                        ho = h * 64
                        pfm_full = pp.tile([128, BLK], F32,
                                           name=f"pfm{layer}_{b}",
                                           tag="fm")
                        if layer == 1:
                            pfm = pfm_full
                            nc.tensor.matmul(
                                out=pfm[:],
                                lhsT=xeown_s[ho:ho + 64,
                                             t * IF:(t + 1) * IF],
                                rhs=diagd_s[ho:ho + 64,
                                            t * BLK:(t + 1) * BLK],
                                start=True, stop=False)
                        else:
                            pfm = pfm_full[:H, :]
                            nc.tensor.matmul(
                                out=pfm[:],
                                lhsT=hs2own_s[ho:ho + 64,
                                              t * H:(t + 1) * H],
                                rhs=diagd_s[ho:ho + 64,
                                            t * BLK:(t + 1) * BLK],
                                start=True, stop=False)
                        for q in range(cfg.NQ):
                            par = q % 2
                            for k in range(cfg.KCOL):
                                lc = bb * cfg.KCOL + k
                                last = (q == cfg.NQ - 1 and
                                        k == cfg.KCOL - 1)
                                if layer == 1:
                                    lhsT_m = msgs[q][:, lc:lc + 1, :]\
                                        .rearrange("p c f -> p (c f)")
                                else:
                                    lhsT_m = msgs[q][:, lc:lc + 1,
                                                     par * H:(par + 1) * H]\
                                        .rearrange("p c f -> p (c f)")
                                rhs_S = Ss[q][:, lc:lc + 1, :]\
                                    .rearrange("p c f -> p (c f)")
                                nc.tensor.matmul(
                                    out=pfm[:], lhsT=lhsT_m, rhs=rhs_S,
                                    start=False, stop=last)
                        if layer == 1:
                            pf_s = hpool.tile([128, BLK], BF16,
                                              name=f"pf1s_{b}", tag="pf1s")
                            nc.vector.tensor_copy(out=pf_s[:], in_=pfm[:])
                            pW = pp.tile([H, BLK], F32, name=f"pW_{b}",
                                         tag="pW")
                            nc.tensor.matmul(
                                out=pW[:], lhsT=w1_s[:], rhs=pf_s[:],
                                start=True, stop=True)
                            hr_t = hpool.tile([H, BLK], BF16,
                                              name=f"hr1_{b}", tag="hr1")
                            nc.scalar.activation(
                                out=hr_t[:], in_=pW[:],
                                func=mybir.ActivationFunctionType.Relu,
                                bias=b1c_s[:])
                            if h == 0:
                                pair["p2"] = pc.tile([128, H], F32,
                                                     name=f"p2_{b}",
                                                     tag="pair")
                            p2 = pair["p2"]
                            nc.tensor.matmul(
                                out=p2[ho:ho + 64, :], lhsT=hr_t[:],
                                rhs=w2_s[:], start=True, stop=True,
                                tile_position=(0, ho))
                            if h == 1:
                                nc.vector.tensor_scalar_mul(
                                    out=hs2own_s[:, t * H:(t + 1) * H],
                                    in0=p2[:],
                                    scalar1=dinvn_s[:, t:t + 1])
                                nc.sync.dma_start(
                                    out=hs2_t[t * 128:(t + 1) * 128, :],
                                    in_=hs2own_s[:, t * H:(t + 1) * H])
                        else:
                            hr_t = hpool.tile([H, BLK], BF16,
                                              name=f"hr2_{b}", tag="hr2")
                            nc.scalar.activation(
                                out=hr_t[:], in_=pfm[:],
                                func=mybir.ActivationFunctionType.Relu,
                                bias=b2c_s[:])
                            if h == 0:
                                pair["pl"] = pc.tile(
                                    [128, cfg.N_CLASSES], F32,
                                    name=f"pl_{b}", tag="pl")
                            pl = pair["pl"]
                            nc.tensor.matmul(
                                out=pl[ho:ho + 64, :], lhsT=hr_t[:],
                                rhs=wl_s[:], start=True, stop=True,
                                tile_position=(0, ho))
                            if h == 1:
                                nCL = cfg.N_CLASSES
                                nc.vector.tensor_tensor(
                                    out=stageL_s[:, t * nCL:(t + 1) * nCL],
                                    in0=pl[:], in1=blrep_s[:],
                                    op=mybir.AluOpType.add)

                # ---- both layers, single AllGather between ----
                with tc.tile_pool(name="xo", bufs=1) as xopool, \
                     tc.tile_pool(name="mp", bufs=3) as mp:
                    xeown_s = xopool.tile([128, NT * IF], BF16,
                                          name="xeown_s", tag="xeown")
                    nc.sync.dma_start(out=xeown_s[:], in_=xeown_d[:, :])
                    for i in range(cfg.N_BATCH):
                        for q in range(cfg.NQ):
                            emit_gather(mp, 1, i, q)
                            emit_S(i, q)
                        emit_consumers(1, i, xeown_s)
                    nc.gpsimd.collective_compute(
                        "AllGather", mybir.AluOpType.bypass,
                        replica_groups=rg,
                        ins=[hs2_t[0:cfg.SLOTS_H, :].opt()],
                        outs=[tabA_t.opt()])
                    nc.gpsimd.collective_compute(
                        "AllGather", mybir.AluOpType.bypass,
                        replica_groups=rg,
                        ins=[hs2_t[cfg.SLOTS_H:cfg.SLOTS_C, :].opt()],
                        outs=[tabB_t.opt()])
                    for i in range(cfg.N_BATCH):
                        for q in range(cfg.NQ):
                            emit_gather(mp, 2, i, q)
                            emit_S(i, q)
                        emit_consumers(2, i, None)

            nc.sync.dma_start(out=logits_d[:, :], in_=stageL_s[:])

    nc.compile()
    return nc


_PROGRAM_CACHE = {}


def get_program(cfg):
    key = id(cfg)
    if key not in _PROGRAM_CACHE:
        _PROGRAM_CACHE[key] = build_program(cfg)
    return _PROGRAM_CACHE[key]


def run(cfg, inputs, trace=False):
    in_maps, node_of_slot = preprocess(cfg, **inputs)
    nc = get_program(cfg)
    res = bass_utils.run_bass_kernel_spmd(
        nc, in_maps, core_ids=list(range(cfg.NC)), trace=trace)
    out = assemble_output(cfg, res.results, node_of_slot)
    return out, res


def kernel(**inputs) -> np.ndarray:
    out, _ = run(CFG_FULL, inputs)
    return out


# revision 24
# speedup vs baseline: 1.0245x; 1.0177x over previous
"""2-layer GCN (GCNConv+relu x2, linear head) on 8 Trainium2 NeuronCores.

v5 strategy (graph/data parallel per the sharding hint):
  - Nodes sharded across 8 cores by id; edges partitioned by destination.
  - Per core, destinations are packed into two HALVES of B_FIX/2 blocks
    each; the 4 gather streams are (source half, source parity), where a
    node's half is fixed a priori by its position within its core. Slot
    tables are laid out HALF-MAJOR across cores ([c0A..c7A, c0B..c7B]) so
    each stream's gather window is contiguous and fits int16 pair indices.
  - Layer 1 gathers raw dinv-scaled x rows (256B = 128 feats bf16) from a
    replicated half-major pair table in HBM; aggregation commutes with
    W1+W2, which are applied per destination block AFTER the scatter-add.
  - The inter-layer AllGather is SPLIT per half: AG_A is issued mid-layer-1
    (its inputs -- the first half of the staging table -- are already
    complete), AG_B at the end; layer-2 A-stream gathers start immediately
    after layer 1 drains, overlapping AG_B (software-pipelined emission).
  - Scatter-add per destination block via PE matmuls: lhsT = gathered
    messages, rhs = selection matrix S with dinv[dst] folded in, built
    host-side and streamed from HBM. Self-loop terms enter via diag(dinv)
    matmuls. The AllGathered staging tables are pair-HBM Shared.
"""

import numpy as np

import concourse.bass as bass
import concourse.mybir as mybir
import concourse.tile as tile
from concourse import bacc
from concourse import bass_utils

import ml_dtypes

F32 = mybir.dt.float32
BF16 = mybir.dt.bfloat16
I16 = mybir.dt.int16
NP_BF16 = ml_dtypes.bfloat16


class Cfg:
    def __init__(self, n_nodes, in_feat, hidden, n_classes, n_cores,
                 n_c, blk, kcol, b_fix, c_batch):
        self.N = n_nodes
        self.IN_FEAT = in_feat
        self.HIDDEN = hidden
        self.N_CLASSES = n_classes
        self.NC = n_cores
        self.N_C = n_c                    # nodes per core (id // N_C)
        assert n_c * n_cores >= n_nodes
        self.BLK = blk                    # dsts per block
        self.KCOL = kcol                  # 128-columns per (block, stream)
        self.CAP = kcol * 128             # edge slots per (block, stream)
        self.B_FIX = b_fix                # blocks per core
        assert b_fix % 2 == 0
        self.B_HALF = b_fix // 2          # blocks per (core, half)
        self.NQ = 4                       # streams = (source half, parity)
        self.SLOTS_C = b_fix * blk        # table slots per core
        self.SLOTS_H = self.SLOTS_C // 2  # slots per (core, half)
        assert self.SLOTS_C % 128 == 0
        self.NT = self.SLOTS_C // 128     # node tiles per core
        assert self.NT % 4 == 0
        self.TABLE_N = n_cores * self.SLOTS_C
        self.WIN = self.TABLE_N // 2      # slots per half-window
        self.PAIRS_W = self.WIN // 2      # pair rows per window
        assert self.PAIRS_W <= 32767      # int16 pair-index range
        self.COLS_Q = b_fix * kcol        # gather columns per stream
        self.C_BATCH = c_batch            # columns per gather call
        assert c_batch % kcol == 0 and self.COLS_Q % c_batch == 0
        self.N_BATCH = self.COLS_Q // c_batch
        self.BPB = c_batch // kcol        # blocks per batch
        assert self.BPB % 2 == 0
        self.N_CH = n_c // 2              # nodes per (core, half)


CFG_FULL = Cfg(n_nodes=100000, in_feat=128, hidden=64, n_classes=16,
               n_cores=8, n_c=12544, blk=64, kcol=2, b_fix=224, c_batch=28)

AG_A_AFTER_BATCH = 13      # emit AG_A after this layer-1 batch
L2_PROLOGUE = 3            # layer-2 A-stream gather batches before AG_B


# ---------------------------------------------------------------------------
# Host-side preprocessing: graph restructuring only.
# ---------------------------------------------------------------------------

def preprocess(cfg, x, edge_index, W1, b1, W2, b2, Wl, bl):
    N, NC, N_C = cfg.N, cfg.NC, cfg.N_C
    src = np.asarray(edge_index[0]).astype(np.int64)
    dst = np.asarray(edge_index[1]).astype(np.int64)
    x = np.asarray(x, dtype=np.float32)

    deg = np.bincount(dst, minlength=N).astype(np.float32) + 1.0
    dinv = (1.0 / np.sqrt(deg)).astype(np.float32)

    # stream of each edge: (source half, source parity)
    q_of = ((src % N_C) >= cfg.N_CH).astype(np.int64) * 2 + (src & 1)

    # per-(node, q) incoming edge counts
    degq = np.bincount(dst * cfg.NQ + q_of, minlength=N * cfg.NQ)\
             .reshape(N, cfg.NQ)

    # --- per-(core,half) FFD packing of dsts into blocks ---
    half = cfg.BLK // 2
    slot_of = np.full(NC * N_C, -1, dtype=np.int64)
    node_of_slot = np.full(cfg.TABLE_N, -1, dtype=np.int64)
    for c in range(NC):
        for hf in range(2):
            lo = c * N_C + hf * cfg.N_CH
            hi = min(lo + cfg.N_CH, N)
            if hi <= lo:
                continue
            b_base = hf * cfg.B_HALF
            dq = degq[lo:hi]
            par = (np.arange(lo, hi) & 1)
            order = np.argsort(-dq.max(axis=1), kind="stable")
            accs = np.zeros((cfg.B_HALF, cfg.NQ), dtype=np.int64)
            cnts = np.zeros((cfg.B_HALF, 2), dtype=np.int64)
            nopen = 1
            for j in order:
                v = dq[j]
                p = par[j]
                fits = (cnts[:nopen, p] < half) & \
                       np.all(accs[:nopen] + v <= cfg.CAP, axis=1)
                w = np.flatnonzero(fits)
                if w.size == 0:
                    assert nopen < cfg.B_HALF, \
                        f"core {c} half {hf}: packing exceeds " \
                        f"{cfg.B_HALF} blocks"
                    b = nopen
                    nopen += 1
                else:
                    b = int(w[0])
                g = lo + j
                s = (c * cfg.SLOTS_C + (b_base + b) * cfg.BLK +
                     2 * cnts[b, p] + p)
                slot_of[g] = s
                node_of_slot[s] = g
                accs[b] += v
                cnts[b, p] += 1

    slot_of = slot_of[:N]

    # --- per-core edge streams ---
    e_core = dst // N_C
    s_slot = slot_of[src]
    d_slot_l = slot_of[dst] - e_core * cfg.SLOTS_C
    e_b = d_slot_l // cfg.BLK
    e_r = d_slot_l % cfg.BLK

    # half-major position of a global slot
    g_all = np.arange(cfg.TABLE_N)
    g_c = g_all // cfg.SLOTS_C
    g_sl = g_all % cfg.SLOTS_C
    g_hf = (g_sl >= cfg.SLOTS_H).astype(np.int64)
    pos_of_slot = (g_hf * cfg.WIN + g_c * cfg.SLOTS_H +
                   (g_sl - g_hf * cfg.SLOTS_H))

    P_Q = cfg.B_FIX * cfg.CAP            # positions per stream
    idx_all = np.zeros((NC, cfg.NQ, P_Q), dtype=np.int16)
    dl_all = np.full((NC, cfg.NQ, P_Q), 255, dtype=np.int64)

    order2 = np.lexsort((e_b, q_of, e_core))
    es_c, eq_c, eb_c = e_core[order2], q_of[order2], e_b[order2]
    grp = (es_c * cfg.NQ + eq_c) * cfg.B_FIX + eb_c
    _, start_idx, cnt_grp = np.unique(grp, return_index=True,
                                      return_counts=True)
    rank = np.arange(grp.size) - np.repeat(start_idx, cnt_grp)
    assert rank.max(initial=0) < cfg.CAP, "stream capacity overflow"
    pos = eb_c * cfg.CAP + rank
    idx_val = ((pos_of_slot[s_slot[order2]] % cfg.WIN) // 2)\
        .astype(np.int16)
    idx_all[es_c, eq_c, pos] = idx_val
    dl_all[es_c, eq_c, pos] = e_r[order2]

    # wrapped int16 layout: position i -> [i%16, i//16], replicated x8
    idx_w = idx_all.reshape(NC, cfg.NQ, -1, 16).transpose(0, 1, 3, 2)
    idx_dev = np.ascontiguousarray(np.tile(idx_w, (1, 1, 8, 1)))

    # --- per-slot node data: x pre-scaled by dinv, slot-ordered ---
    valid = node_of_slot >= 0
    xe = np.zeros((cfg.TABLE_N, cfg.IN_FEAT), dtype=np.float32)
    xe[valid] = x[node_of_slot[valid]] * dinv[node_of_slot[valid]][:, None]
    dinv_s = np.zeros(cfg.TABLE_N, dtype=np.float32)
    dinv_s[valid] = dinv[node_of_slot[valid]]

    # layer-1 gather table in HALF-MAJOR order, pair rows [TN//2, 2*IF]
    xe_hm = np.zeros_like(xe)
    xe_hm[pos_of_slot] = xe
    xe_pair = np.ascontiguousarray(
        xe_hm.reshape(cfg.TABLE_N // 2, 2 * cfg.IN_FEAT)).astype(NP_BF16)

    # S matrices with dinv[dst] folded in: per (core, stream)
    # dev layout [128, COLS_Q*BLK]: S[p, col*BLK + j] for position col*128+p
    S_dev = []
    for c in range(NC):
        r = dl_all[c]                                    # [NQ, P_Q]
        bidx = np.arange(P_Q) // cfg.CAP                 # block of position
        v = r < 255
        dloc = np.where(v, bidx[None, :] * cfg.BLK + r, 0)
        val = dinv_s[c * cfg.SLOTS_C + dloc] * v         # [NQ, P_Q]
        S = np.zeros((cfg.NQ, P_Q, cfg.BLK), dtype=np.float32)
        qq, pp = np.nonzero(v)
        S[qq, pp, r[qq, pp]] = val[qq, pp]
        S = S.reshape(cfg.NQ, cfg.COLS_Q, 128, cfg.BLK)\
             .transpose(0, 2, 1, 3)\
             .reshape(cfg.NQ, 128, cfg.COLS_Q * cfg.BLK)
        S_dev.append(np.ascontiguousarray(S.astype(NP_BF16)))

    # diag(dinv) tiles for self-loop terms: [128, NT*BLK]
    diagd_all = []
    for c in range(NC):
        dv = dinv_s[c * cfg.SLOTS_C:(c + 1) * cfg.SLOTS_C]\
            .reshape(cfg.NT, 128)
        dgd = np.zeros((128, cfg.NT * cfg.BLK), dtype=np.float32)
        t_i = np.arange(cfg.NT)[:, None]
        j_i = np.arange(cfg.BLK)[None, :]
        dgd[j_i, t_i * cfg.BLK + j_i] = dv[:, :64]
        dgd[64 + j_i, t_i * cfg.BLK + j_i] = dv[:, 64:]
        diagd_all.append(dgd.astype(NP_BF16))

    # own xe rows, slot-tile-major for L1 self lhsT: [128, NT*IN_FEAT]
    xe_own_all = []
    for c in range(NC):
        xo = xe[c * cfg.SLOTS_C:(c + 1) * cfg.SLOTS_C]\
            .reshape(cfg.NT, 128, cfg.IN_FEAT).transpose(1, 0, 2)\
            .reshape(128, cfg.NT * cfg.IN_FEAT)
        xe_own_all.append(np.ascontiguousarray(xo).astype(NP_BF16))

    W1 = np.asarray(W1, np.float32).astype(NP_BF16)
    W2 = np.asarray(W2, np.float32).astype(NP_BF16)
    Wl = np.asarray(Wl, np.float32).astype(NP_BF16)
    b1 = np.asarray(b1, np.float32)
    b2 = np.asarray(b2, np.float32)
    bl = np.asarray(bl, np.float32)

    in_maps = []
    for c in range(NC):
        dv = dinv_s[c * cfg.SLOTS_C:(c + 1) * cfg.SLOTS_C]
        m = {
            "xep": xe_pair,
            "xeown": xe_own_all[c],
            "diagd": diagd_all[c],
            "w1": W1, "w2": W2, "wl": Wl,
            "b1c": b1.reshape(-1, 1),
            "b2c": b2.reshape(-1, 1),
            "blrep": np.tile(bl[None, :], (128, 1)),
            "dinvn": np.ascontiguousarray(dv.reshape(cfg.NT, 128).T),
        }
        for q in range(cfg.NQ):
            m[f"idx{q}"] = idx_dev[c, q]
            m[f"sm{q}"] = S_dev[c][q]
        in_maps.append(m)

    return in_maps, node_of_slot


def assemble_output(cfg, results, node_of_slot):
    out = np.zeros((cfg.N, cfg.N_CLASSES), dtype=np.float32)
    for c, r in enumerate(results):
        lg = r["logits"].reshape(128, cfg.NT, cfg.N_CLASSES)
        sl = node_of_slot[c * cfg.SLOTS_C:(c + 1) * cfg.SLOTS_C]\
            .reshape(cfg.NT, 128)
        for t in range(cfg.NT):
            v = sl[t] >= 0
            out[sl[t][v]] = lg[v, t, :]
    return out


# ---------------------------------------------------------------------------
# Device program
# ---------------------------------------------------------------------------

def build_program(cfg):
    nc = bacc.Bacc("TRN2", target_bir_lowering=False, debug=False,
                   num_devices=cfg.NC, num_swdge_queues=4)
    H, NT, BLK, IF = cfg.HIDDEN, cfg.NT, cfg.BLK, cfg.IN_FEAT
    PAIRS_W = cfg.PAIRS_W

    xep_d = nc.dram_tensor("xep", [cfg.TABLE_N // 2, 2 * IF], BF16,
                           kind="ExternalInput")
    xeown_d = nc.dram_tensor("xeown", [128, NT * IF], BF16,
                             kind="ExternalInput")
    diagd_d = nc.dram_tensor("diagd", [128, NT * BLK], BF16,
                             kind="ExternalInput")
    w1_d = nc.dram_tensor("w1", [IF, H], BF16, kind="ExternalInput")
    w2_d = nc.dram_tensor("w2", [H, H], BF16, kind="ExternalInput")
    wl_d = nc.dram_tensor("wl", [H, cfg.N_CLASSES], BF16,
                          kind="ExternalInput")
    b1c_d = nc.dram_tensor("b1c", [H, 1], F32, kind="ExternalInput")
    b2c_d = nc.dram_tensor("b2c", [H, 1], F32, kind="ExternalInput")
    blrep_d = nc.dram_tensor("blrep", [128, cfg.N_CLASSES], F32,
                             kind="ExternalInput")
    dinvn_d = nc.dram_tensor("dinvn", [128, NT], F32, kind="ExternalInput")
    idx_d = [nc.dram_tensor(f"idx{q}", [128, cfg.COLS_Q * 8], I16,
                            kind="ExternalInput") for q in range(cfg.NQ)]
    sm_d = [nc.dram_tensor(f"sm{q}", [128, cfg.COLS_Q * BLK], BF16,
                           kind="ExternalInput") for q in range(cfg.NQ)]
    logits_d = nc.dram_tensor("logits", [128, NT * cfg.N_CLASSES], F32,
                              kind="ExternalOutput")

    rg = [list(range(cfg.NC))]

    with tile.TileContext(nc) as tc:
        with tc.tile_pool(name="const", bufs=1) as cpool, \
             tc.tile_pool(name="dram", bufs=1, space="DRAM") as dpool, \
             tc.tile_pool(name="hp", bufs=3) as hpool:

            hs2_t = dpool.tile([cfg.SLOTS_C, H], BF16, tag="hs2")
            tabA_t = dpool.tile([PAIRS_W, 2 * H], BF16, tag="tabA",
                                addr_space="Shared")
            tabB_t = dpool.tile([PAIRS_W, 2 * H], BF16, tag="tabB",
                                addr_space="Shared")

            def cload(dram, shape, dt, tag):
                t = cpool.tile(shape, dt, name=f"c_{tag}", tag=tag)
                nc.sync.dma_start(out=t[:], in_=dram[:, :])
                return t

            w1_s = cload(w1_d, [IF, H], BF16, "w1")
            w2_s = cload(w2_d, [H, H], BF16, "w2")
            wl_s = cload(wl_d, [H, cfg.N_CLASSES], BF16, "wl")
            b1c_s = cload(b1c_d, [H, 1], F32, "b1c")
            b2c_s = cload(b2c_d, [H, 1], F32, "b2c")
            blrep_s = cload(blrep_d, [128, cfg.N_CLASSES], F32, "blrep")
            dinvn_s = cload(dinvn_d, [128, NT], F32, "dinvn")
            diagd_s = cload(diagd_d, [128, NT * BLK], BF16, "diagd")

            hs2own_s = cpool.tile([128, NT * H], BF16, tag="hs2own")
            stageL_s = cpool.tile([128, NT * cfg.N_CLASSES], F32, tag="stgL")

            with tc.tile_pool(name="sp", bufs=2) as spool, \
                 tc.tile_pool(name="ip", bufs=4) as ipool, \
                 tc.tile_pool(name="pp", bufs=2, space="PSUM") as pp, \
                 tc.tile_pool(name="pc", bufs=2, space="PSUM") as pc:

                gt = {}    # (layer, i, q) -> msg tile
                st = {}    # (i, q) -> S tile

                def emit_gather(mpool, layer, i, q, chunks=1):
                    idx_t = ipool.tile([128, cfg.C_BATCH * 8], I16,
                                       name=f"ix{layer}_{i}_{q}",
                                       tag=f"ix{q}")
                    nc.scalar.dma_start(
                        out=idx_t[:],
                        in_=idx_d[q][:, i * cfg.C_BATCH * 8:
                                     (i + 1) * cfg.C_BATCH * 8])
                    par = q % 2
                    cb = cfg.C_BATCH // chunks
                    if layer == 1:
                        wlo = (q // 2) * PAIRS_W
                        msg_t = mpool.tile([128, cfg.C_BATCH, IF], BF16,
                                           name=f"m1_{i}_{q}",
                                           tag=f"msg{q}")
                        for ch in range(chunks):
                            nc.gpsimd.dma_gather(
                                out_ap=msg_t[:, ch * cb:(ch + 1) * cb, :],
                                in_ap=xep_d[wlo:wlo + PAIRS_W,
                                            par * IF:(par + 1) * IF],
                                idxs_ap=idx_t[:, ch * cb * 8:
                                              (ch + 1) * cb * 8],
                                num_idxs=cb * 128,
                                num_idxs_reg=cb * 128,
                                elem_size=IF, elem_step=2 * IF,
                                queue_num=q, single_packet=False)
                    else:
                        tab = tabA_t if q < 2 else tabB_t
                        msg_t = mpool.tile([128, cfg.C_BATCH, 2 * H], BF16,
                                           name=f"m2_{i}_{q}",
                                           tag=f"msg{q}")
                        for ch in range(chunks):
                            nc.gpsimd.dma_gather(
                                out_ap=msg_t[:, ch * cb:(ch + 1) * cb, :],
                                in_ap=tab[:, :],
                                idxs_ap=idx_t[:, ch * cb * 8:
                                              (ch + 1) * cb * 8],
                                num_idxs=cb * 128,
                                num_idxs_reg=cb * 128,
                                elem_size=2 * H, queue_num=q,
                                single_packet=False)
                    gt[(layer, i, q)] = msg_t

                def emit_S(i, q):
                    S_t = spool.tile([128, cfg.C_BATCH, BLK], BF16,
                                     name=f"S_{i}_{q}", tag=f"S{q}")
                    nc.sync.dma_start(
                        out=S_t[:],
                        in_=sm_d[q][:, i * cfg.C_BATCH * BLK:
                                    (i + 1) * cfg.C_BATCH * BLK]
                        .rearrange("p (c f) -> p c f", f=BLK))
                    st[(i, q)] = S_t

                pair = {}

                def emit_consumers(layer, i, xeown_s):
                    msgs = [gt.pop((layer, i, q)) for q in range(cfg.NQ)]
                    Ss = [st.pop((i, q)) for q in range(cfg.NQ)]
                    for bb in range(cfg.BPB):
                        b = i * cfg.BPB + bb
                        t = b // 2
                        h = b % 2
                        ho = h * 64
                        pfm_full = pp.tile([128, BLK], F32,
                                           name=f"pfm{layer}_{b}",
                                           tag="fm")
                        if layer == 1:
                            pfm = pfm_full
                            nc.tensor.matmul(
                                out=pfm[:],
                                lhsT=xeown_s[ho:ho + 64,
                                             t * IF:(t + 1) * IF],
                                rhs=diagd_s[ho:ho + 64,
                                            t * BLK:(t + 1) * BLK],
                                start=True, stop=False)
                        else:
                            pfm = pfm_full[:H, :]
                            nc.tensor.matmul(
                                out=pfm[:],
                                lhsT=hs2own_s[ho:ho + 64,
                                              t * H:(t + 1) * H],
                                rhs=diagd_s[ho:ho + 64,
                                            t * BLK:(t + 1) * BLK],
                                start=True, stop=False)
                        for q in range(cfg.NQ):
                            par = q % 2
                            for k in range(cfg.KCOL):
                                lc = bb * cfg.KCOL + k
                                last = (q == cfg.NQ - 1 and
                                        k == cfg.KCOL - 1)
                                if layer == 1:
                                    lhsT_m = msgs[q][:, lc:lc + 1, :]\
                                        .rearrange("p c f -> p (c f)")
                                else:
                                    lhsT_m = msgs[q][:, lc:lc + 1,
                                                     par * H:(par + 1) * H]\
                                        .rearrange("p c f -> p (c f)")
                                rhs_S = Ss[q][:, lc:lc + 1, :]\
                                    .rearrange("p c f -> p (c f)")
                                nc.tensor.matmul(
                                    out=pfm[:], lhsT=lhsT_m, rhs=rhs_S,
                                    start=False, stop=last)
                        if layer == 1:
                            pf_s = hpool.tile([128, BLK], BF16,
                                              name=f"pf1s_{b}", tag="pf1s")
                            nc.vector.tensor_copy(out=pf_s[:], in_=pfm[:])
                            pW = pp.tile([H, BLK], F32, name=f"pW_{b}",
                                         tag="pW")
                            nc.tensor.matmul(
                                out=pW[:], lhsT=w1_s[:], rhs=pf_s[:],
                                start=True, stop=True)
                            hr_t = hpool.tile([H, BLK], BF16,
                                              name=f"hr1_{b}", tag="hr1")
                            nc.scalar.activation(
                                out=hr_t[:], in_=pW[:],
                                func=mybir.ActivationFunctionType.Relu,
                                bias=b1c_s[:])
                            if h == 0:
                                pair["p2"] = pc.tile([128, H], F32,
                                                     name=f"p2_{b}",
                                                     tag="pair")
                            p2 = pair["p2"]
                            nc.tensor.matmul(
                                out=p2[ho:ho + 64, :], lhsT=hr_t[:],
                                rhs=w2_s[:], start=True, stop=True,
                                tile_position=(0, ho))
                            if h == 1:
                                nc.vector.tensor_scalar_mul(
                                    out=hs2own_s[:, t * H:(t + 1) * H],
                                    in0=p2[:],
                                    scalar1=dinvn_s[:, t:t + 1])
                                nc.sync.dma_start(
                                    out=hs2_t[t * 128:(t + 1) * 128, :],
                                    in_=hs2own_s[:, t * H:(t + 1) * H])
                        else:
                            hr_t = hpool.tile([H, BLK], BF16,
                                              name=f"hr2_{b}", tag="hr2")
                            nc.scalar.activation(
                                out=hr_t[:], in_=pfm[:],
                                func=mybir.ActivationFunctionType.Relu,
                                bias=b2c_s[:])
                            if h == 0:
                                pair["pl"] = pc.tile(
                                    [128, cfg.N_CLASSES], F32,
                                    name=f"pl_{b}", tag="pl")
                            pl = pair["pl"]
                            nc.tensor.matmul(
                                out=pl[ho:ho + 64, :], lhsT=hr_t[:],
                                rhs=wl_s[:], start=True, stop=True,
                                tile_position=(0, ho))
                            if h == 1:
                                nCL = cfg.N_CLASSES
                                nc.vector.tensor_tensor(
                                    out=stageL_s[:, t * nCL:(t + 1) * nCL],
                                    in0=pl[:], in1=blrep_s[:],
                                    op=mybir.AluOpType.add)

                # ---- both layers, single AllGather between ----
                with tc.tile_pool(name="xo", bufs=1) as xopool, \
                     tc.tile_pool(name="mp", bufs=3) as mp:
                    xeown_s = xopool.tile([128, NT * IF], BF16,
                                          name="xeown_s", tag="xeown")
                    nc.sync.dma_start(out=xeown_s[:], in_=xeown_d[:, :])
                    for i in range(cfg.N_BATCH):
                        for q in range(cfg.NQ):
                            emit_gather(mp, 1, i, q)
                            emit_S(i, q)
                        emit_consumers(1, i, xeown_s)
                    nc.gpsimd.collective_compute(
                        "AllGather", mybir.AluOpType.bypass,
                        replica_groups=rg,
                        ins=[hs2_t[0:cfg.SLOTS_H, :].opt()],
                        outs=[tabA_t.opt()])
                    nc.gpsimd.collective_compute(
                        "AllGather", mybir.AluOpType.bypass,
                        replica_groups=rg,
                        ins=[hs2_t[cfg.SLOTS_H:cfg.SLOTS_C, :].opt()],
                        outs=[tabB_t.opt()])
                    for i in range(cfg.N_BATCH):
                        for q in range(cfg.NQ):
                            emit_gather(mp, 2, i, q)
                            emit_S(i, q)
                        emit_consumers(2, i, None)

            nc.sync.dma_start(out=logits_d[:, :], in_=stageL_s[:])

    nc.compile()
    return nc


_PROGRAM_CACHE = {}


def get_program(cfg):
    key = id(cfg)
    if key not in _PROGRAM_CACHE:
        _PROGRAM_CACHE[key] = build_program(cfg)
    return _PROGRAM_CACHE[key]


def run(cfg, inputs, trace=False):
    in_maps, node_of_slot = preprocess(cfg, **inputs)
    nc = get_program(cfg)
    res = bass_utils.run_bass_kernel_spmd(
        nc, in_maps, core_ids=list(range(cfg.NC)), trace=trace)
    out = assemble_output(cfg, res.results, node_of_slot)
    return out, res


def kernel(**inputs) -> np.ndarray:
    out, _ = run(CFG_FULL, inputs)
    return out
